# revision 6
# baseline (speedup 1.0000x reference)
"""Trainium2 Bass kernel for AstraMambaWrapper (Mamba-1 block over gathered check nodes).

Strategy (8 NeuronCores, tensor-parallel over d_inner = 1024 -> 128 ch/core):
  - Host: gather x_chk = x[seq_idx] ([16384, 512]); each core gets the full
    x_chk (transposed, bf16) plus its 128-channel shard of every weight.
  - Device: software-pipelined over 8 blocks of 2048 timesteps. Per block:
      in_proj (PE) -> causal conv (DVE STT) -> silu via exp/ln chains (ACT)
      x_proj partials (PE) -> per-block AllReduce [64, 2048] (bf16)
      dt = softplus(dtlow @ W_dt + b_dt) (PE + ACT)
      scan: per state n: a = exp(dt*A_n) (ACT), b = dtu*B_n (Pool TT,
        B broadcast via stride-0 DMA), h = tensor_tensor_scan (DVE),
        ys += C_n*h (Pool TT x2); carry copies on ACT.
      y = (ua*D_skip + ys)*silu(z) (DVE STT + Pool TT)
      out_proj partials (PE) -> per-block ReduceScatter [2048,512]->[256,512]
      LayerNorm + residual (DVE/ACT/Pool mix)
    Work is split DVE/Pool/ACT to balance engine busy time; phase1(m+1),
    dt(m+1), out_proj(m-1), collective triggers and LN(m-2) are emitted
    inside block m's state loop so every engine queue stays fed.
  - Host: concat core outputs, scatter back into x.
Degenerate-by-construction params (ln_w=1, ln_b=0) are verified on the host
and baked into the graph; all other params are honored from the inputs.
"""

import os
import sys

sys.path.insert(0, "/opt/trn_rl_repo")

import numpy as np
import ml_dtypes

S = 16384
DM = 512
DI = 1024
DS = 16
RK = 32
DC = 4
NCORE = 8
P = DI // NCORE          # 128 channels per core
TBLK = 2048              # block length (free axis)
NB = S // TBLK           # 8 blocks
SHARE = TBLK // NCORE    # 256 output rows per core per block
SROW = S // NCORE        # 2048 output rows per core
LN_EPS = 1e-5

BF16 = ml_dtypes.bfloat16

_CACHE = {}


def _build(debug=False):
    import concourse.bass as bass
    import concourse.bacc as bacc
    import concourse.mybir as mybir
    import concourse.tile as tile

    f32 = mybir.dt.float32
    bf16 = mybir.dt.bfloat16
    AF = mybir.ActivationFunctionType
    OP = mybir.AluOpType

    nc = bacc.Bacc("TRN2", target_bir_lowering=False, debug=False, num_devices=NCORE)

    # ---- kernel I/O (per-core shards) ----
    xT = nc.dram_tensor("xT", [DM, S], bf16, kind="ExternalInput")           # x_chk.T (replicated)
    wuz = nc.dram_tensor("wuz", [DM, 2 * P], bf16, kind="ExternalInput")     # [:, :P]=u cols, [:, P:]=z cols
    wxp = nc.dram_tensor("wxp", [P, RK + 2 * DS], bf16, kind="ExternalInput")  # cols: dtlow, B0,C0,B1,C1,...
    wdt = nc.dram_tensor("wdt", [RK, P], bf16, kind="ExternalInput")
    wout = nc.dram_tensor("wout", [P, DM], bf16, kind="ExternalInput")
    convw = nc.dram_tensor("convw", [P, DC], f32, kind="ExternalInput")
    smallp = nc.dram_tensor("smallp", [P, 4], f32, kind="ExternalInput")     # conv_b, b_dt, D_skip, -conv_b
    aneg = nc.dram_tensor("aneg", [P, DS], f32, kind="ExternalInput")        # -exp(A_log)
    xres = nc.dram_tensor("xres", [SROW, DM], f32, kind="ExternalInput")
    out = nc.dram_tensor("out", [SROW, DM], f32, kind="ExternalOutput")

    # ---- internal DRAM (per-block collective staging) ----
    dbc_in = [nc.dram_tensor(f"dbc_in{m}", [RK + 2 * DS, TBLK], bf16) for m in range(NB)]
    dbc_out = [nc.dram_tensor(f"dbc_out{m}", [RK + 2 * DS, TBLK], bf16, addr_space="Shared")
               for m in range(NB)]
    op_in = [nc.dram_tensor(f"op_in{m}", [TBLK, DM], bf16) for m in range(NB)]
    op_out = [nc.dram_tensor(f"op_out{m}", [SHARE, DM], bf16) for m in range(NB)]

    rg = [list(range(NCORE))]

    with tile.TileContext(nc) as tc:
        with (
            tc.tile_pool(name="const", bufs=1) as cp,
            tc.tile_pool(name="ubig", bufs=1) as up,
            tc.tile_pool(name="blk", bufs=2) as bp,      # per-block ua/dt
            tc.tile_pool(name="zgp", bufs=2) as zp,      # zg per block
            tc.tile_pool(name="work", bufs=2) as wp,
            tc.tile_pool(name="scan", bufs=2) as sp,
            tc.tile_pool(name="bc", bufs=2) as bcp,      # B/C broadcast pairs
            tc.tile_pool(name="psU", bufs=2, space="PSUM") as psU,
            tc.tile_pool(name="psZ", bufs=2, space="PSUM") as psZ,
            tc.tile_pool(name="psS", bufs=2, space="PSUM") as psS,
            tc.tile_pool(name="psO", bufs=2, space="PSUM") as psO,
        ):
            # ---- constants to SBUF ----
            wuz_sb = cp.tile([128, 4, 2 * P], bf16, tag="wuz")
            nc.sync.dma_start(wuz_sb[:, :, :], wuz.ap().rearrange("(k p) n -> p k n", p=128))
            wxp_sb = cp.tile([P, RK + 2 * DS], bf16, tag="wxp")
            nc.sync.dma_start(wxp_sb[:, :], wxp[:, :])
            wdt_sb = cp.tile([RK, P], bf16, tag="wdt")
            nc.sync.dma_start(wdt_sb[:, :], wdt[:, :])
            wout_sb = cp.tile([P, DM], bf16, tag="wout")
            nc.sync.dma_start(wout_sb[:, :], wout[:, :])
            convw_sb = cp.tile([P, DC], f32, tag="convw")
            nc.sync.dma_start(convw_sb[:, :], convw[:, :])
            smallp_sb = cp.tile([P, 4], f32, tag="smallp")
            nc.sync.dma_start(smallp_sb[:, :], smallp[:, :])
            A_sb = cp.tile([P, DS], f32, tag="A")
            nc.sync.dma_start(A_sb[:, :], aneg[:, :])
            carry = cp.tile([P, DS], f32, tag="carry")
            nc.vector.memset(carry[:, :], 0.0)
            eps_t = cp.tile([P, 1], f32, tag="eps")
            nc.vector.memset(eps_t[:, :], LN_EPS)

            # full u (pre-conv) with 3-col zero head for the causal conv
            u_sb = up.tile([P, S + DC - 1], bf16, tag="u")
            nc.vector.memset(u_sb[:, 0 : DC - 1], 0.0)

            ua_blk = [None] * NB
            dt_blk = [None] * NB
            zg_blk = [None] * NB
            y_blk = [None] * NB

            def emit_p1_piece(m, t4):
                """in_proj + z-gate for 512 columns."""
                lo = m * TBLK + t4 * 512
                xk = wp.tile([128, 4, 512], bf16, tag="xk", name=f"xk_{m}_{t4}")
                nc.sync.dma_start(
                    xk[:, :, :],
                    xT.ap().rearrange("(k p) t -> p k t", p=128)[:, :, lo : lo + 512],
                )
                pu = psU.tile([P, 512], f32, tag="pu")
                pz = psZ.tile([P, 512], f32, tag="pz")
                for k in range(4):
                    nc.tensor.matmul(pu[:, :], lhsT=wuz_sb[:, k, 0:P], rhs=xk[:, k, :],
                                     start=(k == 0), stop=(k == 3))
                for k in range(4):
                    nc.tensor.matmul(pz[:, :], lhsT=wuz_sb[:, k, P : 2 * P], rhs=xk[:, k, :],
                                     start=(k == 0), stop=(k == 3))
                nc.scalar.activation(u_sb[:, DC - 1 + lo : DC - 1 + lo + 512], pu[:, :],
                                     AF.Copy)
                # silu(z) = z*sigmoid(z); sigmoid(z) = exp(-ln(1 + exp(-z)))
                e1 = wp.tile([P, 512], bf16, tag="tg")
                nc.scalar.activation(e1[:, :], pz[:, :], AF.Exp, scale=-1.0)
                l1 = wp.tile([P, 512], bf16, tag="sg")
                nc.scalar.activation(l1[:, :], e1[:, :], AF.Ln, bias=1.0)
                s1 = wp.tile([P, 512], bf16, tag="tg")
                nc.scalar.activation(s1[:, :], l1[:, :], AF.Exp, scale=-1.0)
                if zg_blk[m] is None:
                    zg_blk[m] = zp.tile([P, TBLK], bf16, tag="zg", name=f"zg_{m}")
                nc.vector.tensor_tensor(zg_blk[m][:, t4 * 512 : t4 * 512 + 512],
                                        s1[:, :], pz[:, :], op=OP.mult)

            def emit_conv(m):
                """causal depthwise conv + silu -> ua_blk[m] (DVE + ACT)."""
                lo = m * TBLK
                acc = wp.tile([P, TBLK], bf16, tag="acc", name=f"acc_{m}")
                nc.vector.tensor_scalar(acc[:, :], u_sb[:, lo + DC - 1 : lo + DC - 1 + TBLK],
                                        convw_sb[:, DC - 1 : DC], None, op0=OP.mult)
                for k in range(DC - 2, -1, -1):
                    acc2 = wp.tile([P, TBLK], bf16, tag="acc", name=f"acc_{m}_{k}")
                    nc.vector.scalar_tensor_tensor(
                        acc2[:, :], u_sb[:, lo + k : lo + k + TBLK],
                        convw_sb[:, k : k + 1], acc[:, :], op0=OP.mult, op1=OP.add)
                    acc = acc2
                # silu(acc+cb) = (acc+cb)*exp(-ln(1+exp(-(acc+cb))))
                ua_blk[m] = bp.tile([P, TBLK], bf16, tag="ua", name=f"ua_{m}")
                HB = TBLK // 2
                for hh in range(2):
                    sl = slice(hh * HB, hh * HB + HB)
                    e2 = wp.tile([P, HB], bf16, tag="tgc", name=f"e2_{m}_{hh}")
                    nc.scalar.activation(e2[:, :], acc[:, sl], AF.Exp, scale=-1.0,
                                         bias=smallp_sb[:, 3:4])
                    l2 = wp.tile([P, HB], bf16, tag="tgc2", name=f"l2_{m}_{hh}")
                    nc.scalar.activation(l2[:, :], e2[:, :], AF.Ln, bias=1.0)
                    s2 = wp.tile([P, HB], bf16, tag="tgc", name=f"s2_{m}_{hh}")
                    nc.scalar.activation(s2[:, :], l2[:, :], AF.Exp, scale=-1.0)
                    nc.vector.scalar_tensor_tensor(ua_blk[m][:, sl], acc[:, sl],
                                                   smallp_sb[:, 0:1], s2[:, :],
                                                   op0=OP.add, op1=OP.mult)

            def emit_xproj(m):
                """x_proj partials for block m -> dbc_in[m] (PE + ACT + DMA)."""
                for t4 in range(4):
                    pd = psS.tile([128, 512], f32, tag="sm", name=f"xp_{m}_{t4}")
                    nc.tensor.matmul(pd[0 : RK + 2 * DS, :], lhsT=wxp_sb[:, :],
                                     rhs=ua_blk[m][:, t4 * 512 : t4 * 512 + 512],
                                     start=True, stop=True)
                    de = wp.tile([RK + 2 * DS, 512], bf16, tag="de", name=f"de_{m}_{t4}")
                    nc.scalar.activation(de[:, :], pd[0 : RK + 2 * DS, :], AF.Copy)
                    nc.scalar.dma_start(dbc_in[m][:, t4 * 512 : t4 * 512 + 512], de[:, :])

            def emit_ar(m):
                nc.gpsimd.collective_compute(
                    "AllReduce", OP.add, replica_groups=rg,
                    ins=[dbc_in[m].ap().opt()], outs=[dbc_out[m].ap().opt()])

            def emit_dt(m):
                """dt = softplus(dtlow @ W_dt + b_dt) for block m (PE + ACT)."""
                dl = wp.tile([RK, TBLK], bf16, tag="dl", name=f"dl_{m}")
                nc.sync.dma_start(dl[:, :], dbc_out[m][0:RK, :])
                dt_blk[m] = bp.tile([P, TBLK], bf16, tag="dt", name=f"dt_{m}")
                for t4 in range(4):
                    pt = psS.tile([128, 512], f32, tag="sm", name=f"dt_{m}_{t4}")
                    nc.tensor.matmul(pt[:, :], lhsT=wdt_sb[:, :],
                                     rhs=dl[:, t4 * 512 : t4 * 512 + 512],
                                     start=True, stop=True)
                    ex = wp.tile([P, 512], bf16, tag="ex", name=f"ex_{m}_{t4}")
                    nc.scalar.activation(ex[:, :], pt[:, :], AF.Exp,
                                         bias=smallp_sb[:, 1:2])
                    nc.scalar.activation(dt_blk[m][:, t4 * 512 : t4 * 512 + 512],
                                         ex[:, :], AF.Ln, bias=1.0)

            def emit_outproj_quarter(m, qt):
                """4 out_proj matmuls (512 timesteps) + 1 batched DMA to op_in[m]."""
                ob = wp.tile([128, 4, 512], bf16, tag="ob", name=f"ob_{m}_{qt}")
                for j in range(4):
                    st = qt * 4 + j
                    po = psO.tile([128, DM], f32, tag="po")
                    nc.tensor.matmul(po[:, :],
                                     lhsT=y_blk[m][:, st * 128 : (st + 1) * 128],
                                     rhs=wout_sb[:, :], start=True, stop=True)
                    nc.scalar.activation(ob[:, j, :], po[:, :], AF.Copy)
                og = op_in[m][0:1, 0:1]
                dst = bass.AP(og.tensor, qt * 512 * DM,
                              [[DM, 128], [128 * DM, 4], [1, DM]])
                nc.sync.dma_start(dst, ob[:, :, :])

            def emit_rs(m):
                nc.gpsimd.collective_compute(
                    "ReduceScatter", OP.add, replica_groups=rg,
                    ins=[op_in[m].ap().opt()], outs=[op_out[m].ap().opt()])

            def emit_ln(q):
                """LayerNorm + residual for RS chunk q (2 row-tiles of 128)."""
                for st in range(SHARE // 128):
                    lo = q * SHARE + st * 128
                    r = st * 128
                    yt = wp.tile([128, DM], f32, tag="ln", name=f"yt_{q}_{st}")
                    nc.gpsimd.dma_start(yt[:, :], op_out[q][r : r + 128, :])  # casting DMA
                    musum = wp.tile([128, 1], f32, tag="mu", name=f"mus_{q}_{st}")
                    nc.vector.tensor_reduce(musum[:, :], yt[:, :], axis=mybir.AxisListType.X,
                                            op=OP.add)
                    mu = wp.tile([128, 1], f32, tag="mu2", name=f"mu_{q}_{st}")
                    nc.vector.tensor_scalar(mu[:, :], musum[:, :], 1.0 / DM, None, op0=OP.mult)
                    cent = wp.tile([128, DM], f32, tag="cent", name=f"cent_{q}_{st}")
                    nc.vector.tensor_scalar(cent[:, :], yt[:, :], mu[:, :], None,
                                            op0=OP.subtract)
                    sq = wp.tile([128, DM], f32, tag="ln", name=f"sq_{q}_{st}")
                    varsum = wp.tile([128, 1], f32, tag="vs", name=f"vs_{q}_{st}")
                    nc.scalar.activation(sq[:, :], cent[:, :], AF.Square,
                                         accum_out=varsum[:, :])
                    # rstd = exp(-0.5*ln(var+eps)) - stays in the exp/ln ACT table
                    lv = wp.tile([128, 1], f32, tag="std", name=f"lv_{q}_{st}")
                    nc.scalar.activation(lv[:, :], varsum[:, :], AF.Ln,
                                         bias=eps_t[:, 0:1], scale=1.0 / DM)
                    rstd = wp.tile([128, 1], f32, tag="rstd", name=f"rstd_{q}_{st}")
                    nc.scalar.activation(rstd[:, :], lv[:, :], AF.Exp, scale=-0.5)
                    normed = wp.tile([128, DM], f32, tag="norm", name=f"nrm_{q}_{st}")
                    nc.vector.tensor_scalar(normed[:, :], cent[:, :], rstd[:, :], None,
                                            op0=OP.mult)
                    xr = wp.tile([128, DM], f32, tag="xr", name=f"xr_{q}_{st}")
                    nc.sync.dma_start(xr[:, :], xres[lo : lo + 128, :])
                    of = wp.tile([128, DM], f32, tag="cent", name=f"of_{q}_{st}")
                    nc.gpsimd.tensor_tensor(of[:, :], normed[:, :], xr[:, :], op=OP.add)
                    nc.sync.dma_start(out[lo : lo + 128, :], of[:, :])

            def bcast_pair(m, n):
                """AP reading dbc_out[m] rows (RK+2n, RK+2n+1), each replicated
                across 128 partitions: shape [128, 2, TBLK]."""
                apq = dbc_out[m][RK + 2 * n : RK + 2 * n + 2, :]
                return bass.AP(apq.tensor, apq.offset,
                               [[0, 128], [TBLK, 2], [1, TBLK]])

            # ---- prologue: phase1(0) + AR(0) + dt(0) ----
            for t4 in range(4):
                emit_p1_piece(0, t4)
            emit_conv(0)
            emit_xproj(0)
            emit_ar(0)
            emit_dt(0)

            # ---- main software-pipelined loop ----
            for m in range(NB):
                lo = m * TBLK
                dtu = sp.tile([P, TBLK], bf16, tag="dtu", name=f"dtu_{m}")
                nc.vector.tensor_tensor(dtu[:, :], dt_blk[m][:, :], ua_blk[m][:, :],
                                        op=OP.mult)
                ys = None
                for n in range(DS):
                    bc = bcp.tile([128, 2, TBLK], bf16, tag="bc", name=f"bc_{m}_{n}")
                    nc.sync.dma_start(bc[:, :, :], bcast_pair(m, n))
                    a_t = sp.tile([P, TBLK], bf16, tag="a", name=f"a_{m}_{n}")
                    nc.scalar.activation(a_t[:, :], dt_blk[m][:, :], AF.Exp,
                                         scale=A_sb[:, n : n + 1])
                    b_t = sp.tile([P, TBLK], bf16, tag="b", name=f"b_{m}_{n}")
                    nc.gpsimd.tensor_tensor(b_t[:, :], dtu[:, :], bc[:, 0, :], op=OP.mult)
                    h_t = sp.tile([P, TBLK], bf16, tag="h", name=f"h_{m}_{n}")
                    nc.vector.tensor_tensor_scan(h_t[:, :], a_t[:, :], b_t[:, :],
                                                 initial=carry[:, n : n + 1],
                                                 op0=OP.mult, op1=OP.add)
                    nc.scalar.activation(carry[:, n : n + 1], h_t[:, TBLK - 1 : TBLK],
                                         AF.Copy)
                    yc = sp.tile([P, TBLK], bf16, tag="yc", name=f"yc_{m}_{n}")
                    nc.gpsimd.tensor_tensor(yc[:, :], h_t[:, :], bc[:, 1, :], op=OP.mult)
                    if ys is None:
                        ys = yc
                    else:
                        ys2 = sp.tile([P, TBLK], bf16, tag="ys", name=f"ys_{m}_{n}")
                        nc.gpsimd.tensor_tensor(ys2[:, :], ys[:, :], yc[:, :], op=OP.add)
                        ys = ys2

                    # ---- interleaved future/past work ----
                    if m + 1 < NB:
                        if n < 4:
                            emit_p1_piece(m + 1, n)
                        elif n == 4:
                            emit_conv(m + 1)
                        elif n == 5:
                            emit_xproj(m + 1)
                        elif n == 8:
                            emit_ar(m + 1)
                        elif n == 12:
                            emit_dt(m + 1)
                    if m >= 1 and 8 <= n < 12:
                        emit_outproj_quarter(m - 1, n - 8)
                    if m >= 1 and n == 13:
                        emit_rs(m - 1)
                    if m >= 2 and n == 6:
                        emit_ln(m - 2)

                # epilogue: y = (ua*D_skip + ys) * zg
                tot = sp.tile([P, TBLK], bf16, tag="tot", name=f"tot_{m}")
                nc.vector.scalar_tensor_tensor(tot[:, :], ua_blk[m][:, :],
                                               smallp_sb[:, 2:3], ys[:, :],
                                               op0=OP.mult, op1=OP.add)
                y_blk[m] = sp.tile([P, TBLK], bf16, tag="y", name=f"y_{m}")
                nc.gpsimd.tensor_tensor(y_blk[m][:, :], tot[:, :], zg_blk[m][:, :],
                                        op=OP.mult)

            # ---- tail ----
            for qt in range(4):
                emit_outproj_quarter(NB - 1, qt)
            emit_rs(NB - 1)
            emit_ln(NB - 2)
            emit_ln(NB - 1)

    # All ACT functions used (Exp, Ln, Copy, Square) live in the single
    # "natural_log_exp_and_others" table; restricting the table list stops
    # the load-insertion pass from thrashing between tables.
    import concourse.bacc as bacc_mod
    orig_tables = bacc_mod.get_activation_tables

    def _one_table(arch):
        t = orig_tables(arch)
        return {k: (v if k == "natural_log_exp_and_others" else set()) for k, v in t.items()}

    bacc_mod.get_activation_tables = _one_table
    try:
        nc.compile()
    finally:
        bacc_mod.get_activation_tables = orig_tables
    return nc


def _get_nc():
    if "nc" not in _CACHE:
        _CACHE["nc"] = _build()
    return _CACHE["nc"]


def _make_in_maps(inputs):
    x = np.ascontiguousarray(np.asarray(inputs["x"], dtype=np.float32))
    seq_idx = np.asarray(inputs["seq_idx"], dtype=np.int64)
    W_in = np.asarray(inputs["W_in"], dtype=np.float32)
    conv_w = np.asarray(inputs["conv_w"], dtype=np.float32)
    conv_b = np.asarray(inputs["conv_b"], dtype=np.float32)
    W_xproj = np.asarray(inputs["W_xproj"], dtype=np.float32)
    W_dt = np.asarray(inputs["W_dt"], dtype=np.float32)
    b_dt = np.asarray(inputs["b_dt"], dtype=np.float32)
    A_log = np.asarray(inputs["A_log"], dtype=np.float32)
    D_skip = np.asarray(inputs["D_skip"], dtype=np.float32)
    W_out = np.asarray(inputs["W_out"], dtype=np.float32)
    ln_w = np.asarray(inputs["ln_w"], dtype=np.float32)
    ln_b = np.asarray(inputs["ln_b"], dtype=np.float32)

    # ln scale/bias are identity by construction; they are baked into the graph.
    assert np.allclose(ln_w, 1.0) and np.allclose(ln_b, 0.0), "non-identity LN params unsupported"

    x_chk = x[seq_idx]                              # [S, DM]
    xT = np.ascontiguousarray(x_chk.T).astype(BF16)  # [DM, S]

    # x_proj column order: dtlow(32), then B_n/C_n interleaved pairs
    perm = list(range(RK)) + [c for n in range(DS) for c in (RK + n, RK + DS + n)]

    in_maps = []
    for i in range(NCORE):
        cs = slice(i * P, (i + 1) * P)
        wuz = np.concatenate([W_in[:, cs], W_in[:, DI + i * P : DI + (i + 1) * P]], axis=1)
        in_maps.append({
            "xT": xT,
            "wuz": np.ascontiguousarray(wuz).astype(BF16),
            "wxp": np.ascontiguousarray(W_xproj[cs][:, perm]).astype(BF16),
            "wdt": np.ascontiguousarray(W_dt[:, cs]).astype(BF16),
            "wout": np.ascontiguousarray(W_out[cs]).astype(BF16),
            "convw": np.ascontiguousarray(conv_w[cs]),
            "smallp": np.ascontiguousarray(
                np.stack([conv_b[cs], b_dt[cs], D_skip[cs], -conv_b[cs]],
                         axis=1).astype(np.float32)),
            "aneg": np.ascontiguousarray(-np.exp(A_log[cs]).astype(np.float32)),
            "xres": np.ascontiguousarray(x_chk[_core_rows(i)]),
        })
    return x, seq_idx, in_maps


def _core_rows(i):
    """Absolute check-node indices held by core i's output, in output order."""
    return np.concatenate(
        [np.arange(q * TBLK + i * SHARE, q * TBLK + (i + 1) * SHARE) for q in range(NB)])


def kernel(**inputs):
    from concourse.bass_utils import run_bass_kernel_spmd

    x, seq_idx, in_maps = _make_in_maps(inputs)
    nc = _get_nc()
    trace = bool(int(os.environ.get("KERNEL_TRACE", "0")))
    res = run_bass_kernel_spmd(nc, in_maps, core_ids=list(range(NCORE)), trace=trace)
    if trace:
        _CACHE["last_exec_time_ns"] = res.exec_time_ns
        _CACHE["last_results"] = res
    y = np.empty((S, DM), np.float32)
    for i in range(NCORE):
        y[_core_rows(i)] = np.asarray(res.results[i]["out"])
    outp = x.copy()
    outp[seq_idx] = y
    return outp


# revision 19
# speedup vs baseline: 2.0533x; 2.0533x over previous
"""Trainium2 Bass kernel for AstraMambaWrapper (Mamba-1 block over gathered check nodes).

Strategy (8 NeuronCores, tensor-parallel over d_inner = 1024 -> 128 ch/core):
  - Host: gather x_chk = x[seq_idx] ([16384, 512]); each core gets the full
    x_chk (transposed, bf16) plus its 128-channel shard of every weight.
  - Device: software-pipelined over 8 blocks of 2048 timesteps. Per block:
      in_proj (PE) -> causal conv (DVE STT) -> silu via exp/ln chains (ACT)
      x_proj partials (PE) -> per-block AllReduce [64, 2048] (bf16)
      dt = softplus(dtlow @ W_dt + b_dt) (PE + ACT)
      scan: per state n: a = exp(dt*A_n) (ACT), b = dtu*B_n (Pool TT,
        B broadcast via stride-0 DMA), h = tensor_tensor_scan (DVE),
        ys += C_n*h (Pool TT x2); carry copies on ACT.
      y = (ua*D_skip + ys)*silu(z) (DVE STT + Pool TT)
      out_proj partials (PE) -> per-block ReduceScatter [2048,512]->[256,512]
      LayerNorm + residual (DVE/ACT/Pool mix)
    Work is split DVE/Pool/ACT to balance engine busy time; phase1(m+1),
    dt(m+1), out_proj(m-1), collective triggers and LN(m-2) are emitted
    inside block m's state loop so every engine queue stays fed.
  - Host: concat core outputs, scatter back into x.
Degenerate-by-construction params (ln_w=1, ln_b=0) are verified on the host
and baked into the graph; all other params are honored from the inputs.
"""

import os
import sys

sys.path.insert(0, "/opt/trn_rl_repo")

import numpy as np
import ml_dtypes

S = 16384
DM = 512
DI = 1024
DS = 16
RK = 32
DC = 4
NCORE = 8
P = DI // NCORE          # 128 channels per core
TBLK = 2048              # block length (free axis)
NB = S // TBLK           # 8 blocks
SHARE = TBLK // NCORE    # 256 output rows per core per block
SROW = S // NCORE        # 2048 output rows per core
LN_EPS = 1e-5

BF16 = ml_dtypes.bfloat16

_CACHE = {}


def _build(debug=False):
    import concourse.bass as bass
    import concourse.bacc as bacc
    import concourse.mybir as mybir
    import concourse.tile as tile

    f32 = mybir.dt.float32
    bf16 = mybir.dt.bfloat16
    AF = mybir.ActivationFunctionType
    OP = mybir.AluOpType

    nc = bacc.Bacc("TRN2", target_bir_lowering=False, debug=False, num_devices=NCORE)

    # ---- kernel I/O (per-core shards) ----
    SP3 = S + DC - 1
    xT = nc.dram_tensor("xT", [DM, SP3], bf16, kind="ExternalInput")         # x_chk.T, 3 zero cols at head
    wut = nc.dram_tensor("wut", [DM, DC * P], bf16, kind="ExternalInput")    # W_u cols scaled by conv tap
    wz = nc.dram_tensor("wz", [DM, P], bf16, kind="ExternalInput")
    wxp = nc.dram_tensor("wxp", [P, RK + 2 * DS], bf16, kind="ExternalInput")  # cols: dtlow, B0,C0,B1,C1,...
    wdt = nc.dram_tensor("wdt", [RK, P], bf16, kind="ExternalInput")
    wout = nc.dram_tensor("wout", [P, DM], bf16, kind="ExternalInput")
    ident = nc.dram_tensor("ident", [128, 128], bf16, kind="ExternalInput")
    smallp = nc.dram_tensor("smallp", [P, 4], f32, kind="ExternalInput")     # conv_b, b_dt, D_skip, unused
    aneg = nc.dram_tensor("aneg", [P, DS], f32, kind="ExternalInput")        # -exp(A_log)
    xres = nc.dram_tensor("xres", [SROW, DM], f32, kind="ExternalInput")
    out = nc.dram_tensor("out", [SROW, DM], f32, kind="ExternalOutput")

    # ---- internal DRAM (per-block collective staging) ----
    dbc_in = [nc.dram_tensor(f"dbc_in{m}", [RK + 2 * DS, TBLK], bf16) for m in range(NB)]
    dbc_out = [nc.dram_tensor(f"dbc_out{m}", [RK + 2 * DS, TBLK], bf16, addr_space="Shared")
               for m in range(NB)]
    op_in = [nc.dram_tensor(f"op_in{m}", [TBLK, DM], bf16) for m in range(NB)]
    op_out = [nc.dram_tensor(f"op_out{m}", [SHARE, DM], bf16) for m in range(NB)]

    rg = [list(range(NCORE))]

    with tile.TileContext(nc) as tc:
        with (
            tc.tile_pool(name="const", bufs=1) as cp,
            tc.tile_pool(name="ubig", bufs=1) as up,
            tc.tile_pool(name="blk", bufs=2) as bp,      # per-block ua/dt
            tc.tile_pool(name="zgp", bufs=2) as zp,      # zg per block
            tc.tile_pool(name="work", bufs=2) as wp,
            tc.tile_pool(name="scan", bufs=2) as sp,
            tc.tile_pool(name="bc", bufs=3) as bcp,      # B/C broadcast pairs
            tc.tile_pool(name="psU", bufs=1, space="PSUM") as psU,
            tc.tile_pool(name="psZ", bufs=1, space="PSUM") as psZ,
            tc.tile_pool(name="psS", bufs=1, space="PSUM") as psS,
            tc.tile_pool(name="psO", bufs=1, space="PSUM") as psO,
            tc.tile_pool(name="psY", bufs=1, space="PSUM") as psY,
        ):
            # ---- constants to SBUF ----
            wut_sb = cp.tile([128, 4, DC * P], bf16, tag="wut")
            nc.sync.dma_start(wut_sb[:, :, :], wut.ap().rearrange("(k p) n -> p k n", p=128))
            wz_sb = cp.tile([128, 4, P], bf16, tag="wz")
            nc.sync.dma_start(wz_sb[:, :, :], wz.ap().rearrange("(k p) n -> p k n", p=128))
            id_sb = cp.tile([128, 128], bf16, tag="ident")
            nc.sync.dma_start(id_sb[:, :], ident[:, :])
            wxp_sb = cp.tile([P, RK + 2 * DS], bf16, tag="wxp")
            nc.sync.dma_start(wxp_sb[:, :], wxp[:, :])
            wdt_sb = cp.tile([RK, P], bf16, tag="wdt")
            nc.sync.dma_start(wdt_sb[:, :], wdt[:, :])
            wout_sb = cp.tile([P, DM], bf16, tag="wout")
            nc.sync.dma_start(wout_sb[:, :], wout[:, :])
            smallp_sb = cp.tile([P, 4], f32, tag="smallp")
            nc.sync.dma_start(smallp_sb[:, :], smallp[:, :])
            A_sb = cp.tile([P, DS], f32, tag="A")
            nc.sync.dma_start(A_sb[:, :], aneg[:, :])
            carry = cp.tile([P, DS], f32, tag="carry")
            nc.vector.memset(carry[:, :], 0.0)
            eps_t = cp.tile([P, 1], f32, tag="eps")
            nc.vector.memset(eps_t[:, :], LN_EPS)

            ua_blk = [None] * NB
            dt_blk = [None] * NB
            zg_blk = [None] * NB
            y_blk = [None] * NB

            def emit_p1_piece(m, t4):
                """in_proj (conv folded via per-tap weights) + gates, 512 cols."""
                lo = m * TBLK + t4 * 512
                sl = slice(t4 * 512, t4 * 512 + 512)
                xk = wp.tile([128, 4, 512 + DC - 1], bf16, tag="xk", name=f"xk_{m}_{t4}")
                nc.sync.dma_start(
                    xk[:, :, :],
                    xT.ap().rearrange("(k p) t -> p k t", p=128)[:, :, lo : lo + 512 + DC - 1],
                )
                # uc[c,t] = sum_k sum_tap (W_u*w_tap)[dm,c] x[dm, t-3+tap]
                pu = psU.tile([P, 512], f32, tag="pu")
                pz = psZ.tile([P, 512], f32, tag="pz")
                nmm = 0
                for k in range(4):
                    for tap in range(DC):
                        nc.tensor.matmul(pu[:, :],
                                         lhsT=wut_sb[:, k, tap * P : (tap + 1) * P],
                                         rhs=xk[:, k, tap : tap + 512],
                                         start=(nmm == 0), stop=(nmm == 4 * DC - 1))
                        nmm += 1
                for k in range(4):
                    nc.tensor.matmul(pz[:, :], lhsT=wz_sb[:, k, :],
                                     rhs=xk[:, k, DC - 1 : DC - 1 + 512],
                                     start=(k == 0), stop=(k == 3))
                # ua = silu(uc + conv_b) = w * exp(-ln(1+exp(-w))), w = uc+cb
                w_t = wp.tile([P, 512], bf16, tag="wt", name=f"w_{m}_{t4}")
                nc.scalar.activation(w_t[:, :], pu[:, :], AF.Identity, bias=smallp_sb[:, 0:1])
                e2 = wp.tile([P, 512], bf16, tag="tgc", name=f"e2_{m}_{t4}")
                nc.scalar.activation(e2[:, :], w_t[:, :], AF.Exp, scale=-1.0)
                l2 = wp.tile([P, 512], bf16, tag="tgc2", name=f"l2_{m}_{t4}")
                nc.scalar.activation(l2[:, :], e2[:, :], AF.Ln, bias=1.0)
                s2 = wp.tile([P, 512], bf16, tag="tgc", name=f"s2_{m}_{t4}")
                nc.scalar.activation(s2[:, :], l2[:, :], AF.Exp, scale=-1.0)
                if ua_blk[m] is None:
                    ua_blk[m] = bp.tile([P, TBLK], bf16, tag="ua", name=f"ua_{m}")
                nc.vector.tensor_tensor(ua_blk[m][:, sl], w_t[:, :], s2[:, :], op=OP.mult)
                # silu(z) = z*sigmoid(z); sigmoid(z) = exp(-ln(1 + exp(-z)))
                e1 = wp.tile([P, 512], bf16, tag="tg")
                nc.scalar.activation(e1[:, :], pz[:, :], AF.Exp, scale=-1.0)
                l1 = wp.tile([P, 512], bf16, tag="sg")
                nc.scalar.activation(l1[:, :], e1[:, :], AF.Ln, bias=1.0)
                s1 = wp.tile([P, 512], bf16, tag="tg")
                nc.scalar.activation(s1[:, :], l1[:, :], AF.Exp, scale=-1.0)
                if zg_blk[m] is None:
                    zg_blk[m] = zp.tile([P, TBLK], bf16, tag="zg", name=f"zg_{m}")
                nc.vector.tensor_tensor(zg_blk[m][:, sl], s1[:, :], pz[:, :], op=OP.mult)

            def emit_xproj(m):
                """x_proj partials for block m -> dbc_in[m] (PE + ACT + DMA)."""
                for t4 in range(4):
                    pd = psS.tile([128, 512], f32, tag="sm", name=f"xp_{m}_{t4}")
                    nc.tensor.matmul(pd[0 : RK + 2 * DS, :], lhsT=wxp_sb[:, :],
                                     rhs=ua_blk[m][:, t4 * 512 : t4 * 512 + 512],
                                     start=True, stop=True)
                    de = wp.tile([RK + 2 * DS, 512], bf16, tag="de", name=f"de_{m}_{t4}")
                    nc.scalar.activation(de[:, :], pd[0 : RK + 2 * DS, :], AF.Copy)
                    nc.scalar.dma_start(dbc_in[m][:, t4 * 512 : t4 * 512 + 512], de[:, :])

            def emit_ar(m):
                nc.gpsimd.collective_compute(
                    "AllReduce", OP.add, replica_groups=rg,
                    ins=[dbc_in[m].ap().opt()], outs=[dbc_out[m].ap().opt()])

            def emit_dt(m):
                """dt = softplus(dtlow @ W_dt + b_dt) for block m (PE + ACT)."""
                dl = wp.tile([RK, TBLK], bf16, tag="dl", name=f"dl_{m}")
                nc.sync.dma_start(dl[:, :], dbc_out[m][0:RK, :])
                dt_blk[m] = bp.tile([P, TBLK], bf16, tag="dt", name=f"dt_{m}")
                for t4 in range(4):
                    pt = psS.tile([128, 512], f32, tag="sm", name=f"dt_{m}_{t4}")
                    nc.tensor.matmul(pt[:, :], lhsT=wdt_sb[:, :],
                                     rhs=dl[:, t4 * 512 : t4 * 512 + 512],
                                     start=True, stop=True)
                    ex = wp.tile([P, 512], bf16, tag="ex", name=f"ex_{m}_{t4}")
                    nc.scalar.activation(ex[:, :], pt[:, :], AF.Exp,
                                         bias=smallp_sb[:, 1:2])
                    nc.scalar.activation(dt_blk[m][:, t4 * 512 : t4 * 512 + 512],
                                         ex[:, :], AF.Ln, bias=1.0)

            def emit_outproj_quarter(m, qt):
                """4 out_proj matmuls (512 timesteps) + 1 batched DMA to op_in[m]."""
                ob = wp.tile([128, 4, 512], bf16, tag="ob", name=f"ob_{m}_{qt}")
                for j in range(4):
                    st = qt * 4 + j
                    po = psO.tile([128, DM], f32, tag="po")
                    nc.tensor.matmul(po[:, :],
                                     lhsT=y_blk[m][:, st * 128 : (st + 1) * 128],
                                     rhs=wout_sb[:, :], start=True, stop=True)
                    nc.scalar.activation(ob[:, j, :], po[:, :], AF.Copy)
                og = op_in[m][0:1, 0:1]
                dst = bass.AP(og.tensor, qt * 512 * DM,
                              [[DM, 128], [128 * DM, 4], [1, DM]])
                nc.sync.dma_start(dst, ob[:, :, :])

            def emit_rs(m):
                nc.gpsimd.collective_compute(
                    "ReduceScatter", OP.add, replica_groups=rg,
                    ins=[op_in[m].ap().opt()], outs=[op_out[m].ap().opt()])

            def emit_ln(q):
                """LayerNorm + residual for RS chunk q (2 row-tiles of 128)."""
                for st in range(SHARE // 128):
                    lo = q * SHARE + st * 128
                    r = st * 128
                    yt = wp.tile([128, DM], f32, tag="ln", name=f"yt_{q}_{st}")
                    nc.gpsimd.dma_start(yt[:, :], op_out[q][r : r + 128, :])  # casting DMA
                    musum = wp.tile([128, 1], f32, tag="mu", name=f"mus_{q}_{st}")
                    nc.vector.tensor_reduce(musum[:, :], yt[:, :], axis=mybir.AxisListType.X,
                                            op=OP.add)
                    mu = wp.tile([128, 1], f32, tag="mu2", name=f"mu_{q}_{st}")
                    nc.vector.tensor_scalar(mu[:, :], musum[:, :], 1.0 / DM, None, op0=OP.mult)
                    cent = wp.tile([128, DM], f32, tag="cent", name=f"cent_{q}_{st}")
                    nc.vector.tensor_scalar(cent[:, :], yt[:, :], mu[:, :], None,
                                            op0=OP.subtract)
                    sq = wp.tile([128, DM], f32, tag="ln", name=f"sq_{q}_{st}")
                    varsum = wp.tile([128, 1], f32, tag="vs", name=f"vs_{q}_{st}")
                    nc.scalar.activation(sq[:, :], cent[:, :], AF.Square,
                                         accum_out=varsum[:, :])
                    # rstd = exp(-0.5*ln(var+eps)) - stays in the exp/ln ACT table
                    lv = wp.tile([128, 1], f32, tag="std", name=f"lv_{q}_{st}")
                    nc.scalar.activation(lv[:, :], varsum[:, :], AF.Ln,
                                         bias=eps_t[:, 0:1], scale=1.0 / DM)
                    rstd = wp.tile([128, 1], f32, tag="rstd", name=f"rstd_{q}_{st}")
                    nc.scalar.activation(rstd[:, :], lv[:, :], AF.Exp, scale=-0.5)
                    normed = wp.tile([128, DM], f32, tag="norm", name=f"nrm_{q}_{st}")
                    nc.vector.tensor_scalar(normed[:, :], cent[:, :], rstd[:, :], None,
                                            op0=OP.mult)
                    xr = wp.tile([128, DM], f32, tag="xr", name=f"xr_{q}_{st}")
                    nc.sync.dma_start(xr[:, :], xres[lo : lo + 128, :])
                    of = wp.tile([128, DM], f32, tag="cent", name=f"of_{q}_{st}")
                    nc.gpsimd.tensor_tensor(of[:, :], normed[:, :], xr[:, :], op=OP.add)
                    nc.sync.dma_start(out[lo : lo + 128, :], of[:, :])

            def bcast_pair(m, n):
                """AP reading dbc_out[m] rows (RK+2n, RK+2n+1), each replicated
                across 128 partitions: shape [128, 2, TBLK]."""
                apq = dbc_out[m][RK + 2 * n : RK + 2 * n + 2, :]
                return bass.AP(apq.tensor, apq.offset,
                               [[0, 128], [TBLK, 2], [1, TBLK]])

            # ---- prologue: phase1(0) + AR(0) + dt(0) ----
            for t4 in range(4):
                emit_p1_piece(0, t4)
            emit_xproj(0)
            emit_ar(0)
            emit_dt(0)

            # ---- main software-pipelined loop ----
            for m in range(NB):
                lo = m * TBLK
                dtu = sp.tile([P, TBLK], bf16, tag="dtu", name=f"dtu_{m}")
                nc.vector.tensor_tensor(dtu[:, :], dt_blk[m][:, :], ua_blk[m][:, :],
                                        op=OP.mult)
                ys_ps = psY.tile([128, TBLK], f32, tag="ys", name=f"ysp_{m}")
                for n in range(DS):
                    bc = bcp.tile([128, 2, TBLK], bf16, tag="bc", name=f"bc_{m}_{n}")
                    nc.sync.dma_start(bc[:, :, :], bcast_pair(m, n))
                    a_t = sp.tile([P, TBLK], bf16, tag="a", name=f"a_{m}_{n}")
                    nc.scalar.activation(a_t[:, :], dt_blk[m][:, :], AF.Exp,
                                         scale=A_sb[:, n : n + 1])
                    b_t = sp.tile([P, TBLK], bf16, tag="b", name=f"b_{m}_{n}")
                    nc.vector.tensor_tensor(b_t[:, :], dtu[:, :], bc[:, 0, :], op=OP.mult)
                    h_t = sp.tile([P, TBLK], bf16, tag="h", name=f"h_{m}_{n}")
                    nc.vector.tensor_tensor_scan(h_t[:, :], a_t[:, :], b_t[:, :],
                                                 initial=carry[:, n : n + 1],
                                                 op0=OP.mult, op1=OP.add)
                    nc.scalar.activation(carry[:, n : n + 1], h_t[:, TBLK - 1 : TBLK],
                                         AF.Copy)
                    yc = sp.tile([P, TBLK], bf16, tag="yc", name=f"yc_{m}_{n}")
                    nc.vector.tensor_tensor(yc[:, :], h_t[:, :], bc[:, 1, :], op=OP.mult)
                    # ys += yc, accumulated on PE via identity matmul into PSUM
                    # (one matmul per 512-col PSUM bank)
                    for j in range(TBLK // 512):
                        nc.tensor.matmul(ys_ps[:, j * 512 : (j + 1) * 512],
                                         lhsT=id_sb[:, :],
                                         rhs=yc[:, j * 512 : (j + 1) * 512],
                                         start=(n == 0), stop=(n == DS - 1))

                    # ---- interleaved future/past work ----
                    if m + 1 < NB:
                        if n < 4:
                            emit_p1_piece(m + 1, n)
                        elif n == 5:
                            emit_xproj(m + 1)
                        elif n == 8:
                            emit_ar(m + 1)
                        elif n == 12:
                            emit_dt(m + 1)
                    if m >= 1 and 8 <= n < 12:
                        emit_outproj_quarter(m - 1, n - 8)
                    if m >= 1 and n == 13:
                        emit_rs(m - 1)
                    if m >= 2 and n == 6:
                        emit_ln(m - 2)

                # epilogue: y = (ua*D_skip + ys) * zg
                tot = sp.tile([P, TBLK], bf16, tag="tot", name=f"tot_{m}")
                nc.vector.scalar_tensor_tensor(tot[:, :], ua_blk[m][:, :],
                                               smallp_sb[:, 2:3], ys_ps[:, :],
                                               op0=OP.mult, op1=OP.add)
                y_blk[m] = sp.tile([P, TBLK], bf16, tag="y", name=f"y_{m}")
                nc.vector.tensor_tensor(y_blk[m][:, :], tot[:, :], zg_blk[m][:, :],
                                        op=OP.mult)

            # ---- tail ----
            for qt in range(4):
                emit_outproj_quarter(NB - 1, qt)
            emit_rs(NB - 1)
            emit_ln(NB - 2)
            emit_ln(NB - 1)

    # All ACT functions used (Exp, Ln, Copy, Square) live in the single
    # "natural_log_exp_and_others" table; restricting the table list stops
    # the load-insertion pass from thrashing between tables.
    import concourse.bacc as bacc_mod
    orig_tables = bacc_mod.get_activation_tables

    def _one_table(arch):
        t = orig_tables(arch)
        return {k: (v if k == "natural_log_exp_and_others" else set()) for k, v in t.items()}

    bacc_mod.get_activation_tables = _one_table
    try:
        nc.compile()
    finally:
        bacc_mod.get_activation_tables = orig_tables
    return nc


def _get_nc():
    if "nc" not in _CACHE:
        _CACHE["nc"] = _build()
    return _CACHE["nc"]


def _make_in_maps(inputs):
    x = np.ascontiguousarray(np.asarray(inputs["x"], dtype=np.float32))
    seq_idx = np.asarray(inputs["seq_idx"], dtype=np.int64)
    W_in = np.asarray(inputs["W_in"], dtype=np.float32)
    conv_w = np.asarray(inputs["conv_w"], dtype=np.float32)
    conv_b = np.asarray(inputs["conv_b"], dtype=np.float32)
    W_xproj = np.asarray(inputs["W_xproj"], dtype=np.float32)
    W_dt = np.asarray(inputs["W_dt"], dtype=np.float32)
    b_dt = np.asarray(inputs["b_dt"], dtype=np.float32)
    A_log = np.asarray(inputs["A_log"], dtype=np.float32)
    D_skip = np.asarray(inputs["D_skip"], dtype=np.float32)
    W_out = np.asarray(inputs["W_out"], dtype=np.float32)
    ln_w = np.asarray(inputs["ln_w"], dtype=np.float32)
    ln_b = np.asarray(inputs["ln_b"], dtype=np.float32)

    # ln scale/bias are identity by construction; they are baked into the graph.
    assert np.allclose(ln_w, 1.0) and np.allclose(ln_b, 0.0), "non-identity LN params unsupported"

    x_chk = x[seq_idx]                              # [S, DM]
    xTp = np.zeros((DM, S + DC - 1), np.float32)    # 3 zero cols at head for causal conv
    xTp[:, DC - 1 :] = x_chk.T
    xTp = np.ascontiguousarray(xTp).astype(BF16)

    # x_proj column order: dtlow(32), then B_n/C_n interleaved pairs
    perm = list(range(RK)) + [c for n in range(DS) for c in (RK + n, RK + DS + n)]
    ident = np.eye(128, dtype=np.float32).astype(BF16)

    in_maps = []
    for i in range(NCORE):
        cs = slice(i * P, (i + 1) * P)
        # conv folded into in_proj: tap k weight = W_u[:, c] * conv_w[c, k]
        wut = np.concatenate([W_in[:, cs] * conv_w[cs, k][None, :] for k in range(DC)],
                             axis=1)                # [DM, DC*P]
        in_maps.append({
            "xT": xTp,
            "wut": np.ascontiguousarray(wut).astype(BF16),
            "wz": np.ascontiguousarray(W_in[:, DI + i * P : DI + (i + 1) * P]).astype(BF16),
            "wxp": np.ascontiguousarray(W_xproj[cs][:, perm]).astype(BF16),
            "wdt": np.ascontiguousarray(W_dt[:, cs]).astype(BF16),
            "wout": np.ascontiguousarray(W_out[cs]).astype(BF16),
            "ident": ident,
            "smallp": np.ascontiguousarray(
                np.stack([conv_b[cs], b_dt[cs], D_skip[cs], -conv_b[cs]],
                         axis=1).astype(np.float32)),
            "aneg": np.ascontiguousarray(-np.exp(A_log[cs]).astype(np.float32)),
            "xres": np.ascontiguousarray(x_chk[_core_rows(i)]),
        })
    return x, seq_idx, in_maps


def _core_rows(i):
    """Absolute check-node indices held by core i's output, in output order."""
    return np.concatenate(
        [np.arange(q * TBLK + i * SHARE, q * TBLK + (i + 1) * SHARE) for q in range(NB)])


def kernel(**inputs):
    from concourse.bass_utils import run_bass_kernel_spmd

    x, seq_idx, in_maps = _make_in_maps(inputs)
    nc = _get_nc()
    trace = bool(int(os.environ.get("KERNEL_TRACE", "0")))
    res = run_bass_kernel_spmd(nc, in_maps, core_ids=list(range(NCORE)), trace=trace)
    if trace:
        _CACHE["last_exec_time_ns"] = res.exec_time_ns
        _CACHE["last_results"] = res
    y = np.empty((S, DM), np.float32)
    for i in range(NCORE):
        y[_core_rows(i)] = np.asarray(res.results[i]["out"])
    outp = x.copy()
    outp[seq_idx] = y
    return outp
